# revision 36
# baseline (speedup 1.0000x reference)
"""Enformer dot-product self-attention with central-mask relative position
bias, on 8 Trainium2 NeuronCores (one head per core, SPMD).

Math per head h (S=2048, D=64, N=64):
    basis[i,j,:] = f(d=i-j)  — indicator features, zero for |d| > 1024
    logits = (q @ k^T + (q @ w) @ basis^T + u @ k^T + (v @ w) @ basis^T) / 8
    out    = softmax(logits) @ value

Device formulation per core:
  - qT_aug [65,S]: rows 0..63 = (q/8)^T, row 64 = ones.
  - k_augT [65,S]: rows 0..63 = k^T (loaded), row 64 = (u/8) @ k^T (computed).
    qk+uk logits tile = qT_aug_tile^T @ k_augT  (PE, fp32r).
  - Band term: T[i,c'] = qT_aug[:,i] . w2r[:,c'] where c' = j-i+1024 in
    [0,2048], w2r[65, 2176] host-built from w (rows 0..63) and v@w/8 (row 64),
    columns >= 2049 are zero padding.  T is written to DRAM [S rows, pitch
    2176]; the diagonal (rel-shift) read back uses a flat access pattern with
    row step 2175 so each partition p lands j-aligned; out-of-band elements
    read the zero padding of the previous row.
  - Segment A (all 16 tiles up front): band matmuls -> PSUM -> SBUF bf16
    (evac alternating scalar/vector) -> DRAM G_t (gpsimd queue).  Keeps the
    PE stream dense and the scalar/vector engines free for segment C.
  - Segment C per i-tile, quarter-granularity (512 key columns): skewed
    G-read quarter (scalar queue) || qk matmul quarter (fp32r) -> DVE add
    -> ScalarE exp -> P bf16.  No max subtraction (logit range is small).
  - P^T via the DMA XBAR (dma_start_transpose, one call per i-tile with a
    3D out AP scattering 16 [128,128] blocks); XBARs own the sync queue.
  - attn@v lags two tiles (av(t-2)) so its XBAR is long done: out_aug
    [65, 128] = sum_jb [v_jb | 1]^T @ P^T_jb; row 64 accumulates the
    softmax denominators for free.  F(t-3) lags one more: PE transpose of
    the out_aug chunk, 1/denom scale (DVE), store.
"""

import numpy as np
import ml_dtypes

import concourse.bass as bass
import concourse.bacc as bacc
import concourse.mybir as mybir
import concourse.tile as tile
from concourse.bass_utils import run_bass_kernel_spmd
from concourse.masks import make_identity

S = 2048
D = 64
NB = 64          # pos-emb dim (basis features)
H = 8
HALF = NB // 2   # 32
BAND = 1024      # max |d| with nonzero features
Q = S + 128      # G row pitch (2049 band cols + 127 zero pad)
F32 = mybir.dt.float32
F32R = mybir.dt.float32r
BF16 = mybir.dt.bfloat16

_NC_CACHE = {}


def _basis_feature_matrix():
    """Rr[c', n] for c' in [0, Q): features of distance d = 1024 - c'.
    Matches reference._relative_basis numerics (float32)."""
    pow_rate = np.float32(np.exp(np.log((S + 1) / 2) / HALF))
    widths = np.power(pow_rate, np.arange(1, HALF + 1, dtype=np.float32),
                      dtype=np.float32)  # [32]
    d = (np.float32(BAND) - np.arange(Q, dtype=np.float32))[:, None]  # [Q,1]
    unsigned = (np.abs(d) <= widths[None, :]).astype(np.float32)      # [Q,32]
    signed = np.sign(d) * unsigned
    return np.concatenate([unsigned, signed], axis=-1)  # [Q, 64]


def _build_nc():
    if "nc" in _NC_CACHE:
        return _NC_CACHE["nc"]

    nc = bacc.Bacc("TRN2", target_bir_lowering=False, debug=False,
                   num_devices=H)
    d_kT = nc.dram_tensor("kT", [D, S], BF16, kind="ExternalInput")
    d_u8 = nc.dram_tensor("u8", [D, 1], BF16, kind="ExternalInput")
    d_v = nc.dram_tensor("vb", [S, D + 1], BF16, kind="ExternalInput")
    d_w2r = nc.dram_tensor("w2r", [65, Q], BF16, kind="ExternalInput")
    d_qTb = nc.dram_tensor("qT_bf", [65, S], BF16, kind="ExternalInput")
    d_out = nc.dram_tensor("out", [S, D], F32, kind="ExternalOutput")

    NT = S // 128  # 16 i/j tiles
    d_Gs = [nc.dram_tensor(f"gband{t}", [128 * Q], BF16, kind="Internal")
            for t in range(NT)]

    with tile.TileContext(nc) as tc:
        with tc.tile_pool(name="pers", bufs=1) as pers:
            # Queue roles: sync = G writes + output writes (all async
            # issues); scalar = exp + initial loads + share of A evacs;
            # vector/gpsimd = adds, evacs, G-read issues (gpsimd).
            # All matmul operands are zero-padded to K=128 (bf16 K=128
            # streams ~0.5 ns/col vs ~0.84 at K=65).
            sb_qTb = pers.tile([128, S], BF16)
            sb_w2r = pers.tile([128, Q], BF16)
            nc.gpsimd.memset(sb_w2r[64:128, :], 0.0)
            nc.gpsimd.memset(sb_qTb[64:128, :], 0.0)
            nc.scalar.dma_start(out=sb_qTb[0:65, 0:128], in_=d_qTb[:, 0:128])
            # A(0)'s three matmuls need cols [897,1409), [1409,1921),
            # [1921,2176) in that order; A(t) then extends down to
            # clo = 897-128t.  Load in exactly that consumption order.
            for lo, hi in ((897, 1409), (1409, 1921), (1921, Q),
                           (385, 897), (0, 385)):
                nc.scalar.dma_start(out=sb_w2r[0:65, lo:hi],
                                    in_=d_w2r[:, lo:hi])
            nc.scalar.dma_start(out=sb_qTb[0:65, 128:640],
                                in_=d_qTb[:, 128:640])
            nc.scalar.dma_start(out=sb_qTb[0:65, 640:1344],
                                in_=d_qTb[:, 640:1344])
            nc.scalar.dma_start(out=sb_qTb[0:65, 1344:S],
                                in_=d_qTb[:, 1344:S])
            sb_kaug = pers.tile([128, S], BF16)
            nc.gpsimd.memset(sb_kaug[64:128, :], 0.0)
            for c in range(4):
                nc.gpsimd.dma_start(out=sb_kaug[0:D, c * 512:(c + 1) * 512],
                                    in_=d_kT[:, c * 512:(c + 1) * 512])
            sb_v = pers.tile([128, NT, D + 1], BF16)   # [v | ones]
            sb_id = pers.tile([128, 128], F32)
            sb_idb = pers.tile([128, 128], BF16)
            sb_P = pers.tile([128, NT, S], BF16)      # exp(logits), i-tiled
            sb_oT = pers.tile([D + 1, S], F32)        # out^T + denom row
            sb_u8 = pers.tile([D, 1], BF16)
            nc.gpsimd.dma_start(out=sb_u8[:], in_=d_u8[:])

            QW = 512  # quarter width (key columns per qk/exp step)
            with tc.tile_pool(name="gsb", bufs=6) as gsb, \
                 tc.tile_pool(name="bandsb", bufs=3) as bsb, \
                 tc.tile_pool(name="ptsb", bufs=4) as ptsb, \
                 tc.tile_pool(name="fsb", bufs=3) as fsb, \
                 tc.tile_pool(name="ps", bufs=1, space="PSUM") as ps:

                def phase_A(t):
                    i0 = t * 128
                    jlo = max(0, i0 - BAND)
                    jhi = min(S, i0 + 128 + BAND)
                    clo = max(0, (jlo - i0 + BAND) - 127)
                    chi = min(2049, (jhi - 1) - i0 + BAND + 1)
                    gt = gsb.tile([128, Q], BF16)
                    nc.gpsimd.memset(gt[:, chi:Q], 0.0)
                    cuts = list(range(clo, chi, 512)) + [chi]
                    for ci in range(len(cuts) - 1):
                        lo, hi = cuts[ci], cuts[ci + 1]
                        pg = ps.tile([128, 512], F32, tag="c1", bufs=3)
                        nc.tensor.matmul(
                            pg[:, 0:hi - lo],
                            lhsT=sb_qTb[:, i0:i0 + 128],
                            rhs=sb_w2r[:, lo:hi],
                            start=True, stop=True)
                        if (t + ci) % 2 == 0:
                            nc.scalar.copy(out=gt[:, lo:hi],
                                           in_=pg[:, 0:hi - lo])
                        else:
                            nc.vector.tensor_copy(gt[:, lo:hi],
                                                  pg[:, 0:hi - lo])
                    wr = bass.AP(tensor=d_Gs[t], offset=clo,
                                 ap=[[Q, 128], [1, Q - clo]])
                    nc.sync.dma_start(out=wr, in_=gt[:, clo:Q])

                def phase_Crd(t):
                    # Skewed G read for tile t (single wide DMA: 128
                    # descriptors of up to 4.3KB); issued from the gpsimd
                    # queue 2 tiles ahead of use so the transfer hides
                    # under two tiles of exp work.
                    i0 = t * 128
                    jlo = max(0, i0 - BAND)
                    jhi = min(S, i0 + 128 + BAND)
                    bt = bsb.tile([128, Q], BF16)
                    wdt = jhi - jlo
                    rd = bass.AP(tensor=d_Gs[t], offset=(jlo - i0 + BAND),
                                 ap=[[Q - 1, 128], [1, wdt]])
                    nc.gpsimd.dma_start(out=bt[:, 0:wdt], in_=rd)
                    return bt

                def phase_C(t, bt):
                    i0 = t * 128
                    jlo = max(0, i0 - BAND)
                    jhi = min(S, i0 + 128 + BAND)
                    for hf in range(2):
                        j0 = hf * 1024
                        pq = ps.tile([128, 1024], F32, tag="qk", bufs=2)
                        for c2 in range(2):
                            nc.tensor.matmul(
                                pq[:, c2 * QW:(c2 + 1) * QW],
                                lhsT=sb_qTb[:, i0:i0 + 128],
                                rhs=sb_kaug[:, j0 + c2 * QW:j0 + (c2 + 1) * QW],
                                start=True, stop=True)
                        alo = max(jlo, j0)
                        ahi = min(jhi, j0 + 1024)
                        if alo < ahi:
                            nc.vector.tensor_add(
                                pq[:, alo - j0:ahi - j0],
                                pq[:, alo - j0:ahi - j0],
                                bt[:, alo - jlo:ahi - jlo])
                        nc.scalar.activation(
                            out=sb_P[:, t, j0:j0 + 1024], in_=pq[:],
                            func=mybir.ActivationFunctionType.Exp)

                po_live = {}

                def phase_E_sub(c, sub):
                    # Quarter of chunk c's attn@v: PE-transpose P blocks
                    # for key-block range [4*sub, 4*sub+4) (bf16 PSUM out),
                    # evacuate (alternating scalar/vector), accumulate
                    # out_aug[65, 512] over the 16 key blocks.
                    if sub == 0:
                        po_live[c] = ps.tile([D + 1, 512], F32, tag="po",
                                             bufs=1, name=f"po{c}")
                    po = po_live[c]
                    for jp in range(2):  # pairs of key blocks
                        jb0 = 4 * sub + 2 * jp
                        ptp = ps.tile([128, 1024], BF16, tag="c1", bufs=3)
                        for jo in range(2):
                            for s in range(4):
                                t = 4 * c + s
                                nc.tensor.transpose(
                                    ptp[:, jo * 512 + s * 128:
                                        jo * 512 + (s + 1) * 128],
                                    sb_P[:, t, (jb0 + jo) * 128:
                                         (jb0 + jo + 1) * 128],
                                    sb_idb[:])
                        pt = ptsb.tile([128, 1024], BF16)
                        if (sub + jp) % 2 == 0:
                            nc.scalar.copy(out=pt[:], in_=ptp[:])
                        else:
                            nc.vector.tensor_copy(pt[:], ptp[:])
                        for jo in range(2):
                            jb = jb0 + jo
                            nc.tensor.matmul(po[:], lhsT=sb_v[:, jb, :],
                                             rhs=pt[:, jo * 512:
                                                     (jo + 1) * 512],
                                             start=(jb == 0),
                                             stop=(jb == NT - 1))
                    if sub == 3:
                        nc.vector.tensor_copy(
                            sb_oT[:, c * 512:(c + 1) * 512],
                            po_live.pop(c))

                def phase_F(t):
                    # pf borrows a "qk" slot (same 2KB size) so the po
                    # accumulator bank is never blocked by output fixup.
                    pf = ps.tile([128, 512], F32, tag="qk", bufs=2)
                    nc.tensor.transpose(pf[:, 0:D + 1],
                                        sb_oT[:, t * 128:(t + 1) * 128],
                                        sb_id[0:D + 1, 0:D + 1])
                    rc = fsb.tile([128, 1], F32, tag="rc")
                    nc.vector.reciprocal(rc[:], pf[:, D:D + 1])
                    ot = fsb.tile([128, D], F32, tag="ot")
                    nc.vector.tensor_scalar_mul(ot[:], pf[:, 0:D], rc[:])
                    nc.sync.dma_start(out=d_out[t * 128:(t + 1) * 128, :],
                                      in_=ot[:])

                # ---- Segment A: all band tiles up front (dense PE);
                #      first two G reads prefetched mid-segment. ----
                bts = {}
                for t in range(NT):
                    phase_A(t)
                    if t == 2:
                        bts[0] = phase_Crd(0)
                    if t == 3:
                        bts[1] = phase_Crd(1)
                # Deferred setup (needed from C(0)/E(0) on).
                for c in range(4):
                    pk = ps.tile([128, 512], F32, tag="c1", bufs=3)
                    nc.tensor.matmul(
                        pk[0:1, :],
                        lhsT=sb_u8[:],
                        rhs=sb_kaug[0:D, c * 512:(c + 1) * 512],
                        start=True, stop=True)
                    nc.scalar.copy(
                        out=sb_kaug[64:65, c * 512:(c + 1) * 512],
                        in_=pk[0:1, :])
                for t in range(NT):
                    nc.gpsimd.dma_start(out=sb_v[:, t, :],
                                        in_=d_v[t * 128:(t + 1) * 128, :])
                make_identity(nc, sb_id[:])
                make_identity(nc, sb_idb[:])

                # ---- Segment C: qk + band add + exp per tile; attn@v
                #      (E) spread one key-quarter per iteration, lagging
                #      4-7 tiles behind its chunk's exps; F per tile. ----
                for t in range(NT):
                    if t + 2 < NT:
                        bts[t + 2] = phase_Crd(t + 2)
                    phase_C(t, bts.pop(t))
                    if t - 4 >= 0:
                        phase_E_sub((t - 4) // 4, (t - 4) % 4)
                    if t - 8 >= 0:
                        phase_F(t - 8)
                for k in range(4):
                    phase_E_sub(3, k)
                    phase_F(NT - 8 + k)
                for t in range(NT - 4, NT):
                    phase_F(t)

    nc.finalize()
    _NC_CACHE["nc"] = nc
    return nc


def _host_prep(query, key, value, u, v, w):
    """Build the 8 per-core input maps from the full inputs."""
    q = np.asarray(query, np.float32)[0]   # [S,H,D]
    k = np.asarray(key, np.float32)[0]
    val = np.asarray(value, np.float32)[0]
    u = np.asarray(u, np.float32)
    v = np.asarray(v, np.float32)
    w = np.asarray(w, np.float32)
    Rr = _basis_feature_matrix()           # [Q, 64]

    ones = np.ones((1, S), np.float32)
    in_maps = []
    for h in range(H):
        qT8 = np.ascontiguousarray(q[:, h, :].T) / np.float32(8.0)  # [64,S]
        qT_aug = np.concatenate([qT8, ones], axis=0)                # [65,S]
        kT = np.ascontiguousarray(k[:, h, :].T)                     # [64,S]
        u8 = (u[h] / np.float32(8.0)).reshape(D, 1)
        vb = np.concatenate([val[:, h, :], ones.T], axis=1)         # [S,65]
        vb = vb.astype(ml_dtypes.bfloat16)
        w2r_qr = w[h] @ Rr.T                                        # [64,Q]
        vw8 = (v[h] @ w[h]) / np.float32(8.0)                       # [64]
        w2r_vr = (vw8 @ Rr.T).reshape(1, Q)                         # [1,Q]
        w2r = np.concatenate([w2r_qr, w2r_vr],
                             axis=0).astype(ml_dtypes.bfloat16)
        in_maps.append({
            "qT_bf": np.ascontiguousarray(qT_aug).astype(ml_dtypes.bfloat16),
            "kT": kT.astype(ml_dtypes.bfloat16),
            "u8": np.ascontiguousarray(u8).astype(ml_dtypes.bfloat16),
            "vb": np.ascontiguousarray(vb),
            "w2r": np.ascontiguousarray(w2r),
        })
    return in_maps


def kernel(query, key, value, u, v, w, _trace=False):
    nc = _build_nc()
    in_maps = _host_prep(query, key, value, u, v, w)
    res = run_bass_kernel_spmd(nc, in_maps, core_ids=list(range(H)),
                               trace=_trace)
    outs = np.stack([res.results[h]["out"] for h in range(H)])  # [H,S,D]
    full = np.transpose(outs, (1, 0, 2))[None]                  # [1,S,H,D]
    out = np.ascontiguousarray(full.astype(np.float32))
    if _trace:
        return out, res
    return out


if __name__ == "__main__":
    rng = np.random.default_rng(0)
    ins = {
        "query": rng.standard_normal((1, S, H, D), np.float32),
        "key": rng.standard_normal((1, S, H, D), np.float32),
        "value": rng.standard_normal((1, S, H, D), np.float32),
        "u": rng.standard_normal((H, D), np.float32),
        "v": rng.standard_normal((H, D), np.float32),
        "w": rng.standard_normal((H, D, NB), np.float32),
    }
    out = kernel(**ins)
    print("out shape:", out.shape, "finite:", np.isfinite(out).all())


# revision 37
# speedup vs baseline: 1.0639x; 1.0639x over previous
"""Enformer dot-product self-attention with central-mask relative position
bias, on 8 Trainium2 NeuronCores (one head per core, SPMD).

Math per head h (S=2048, D=64, N=64):
    basis[i,j,:] = f(d=i-j)  — indicator features, zero for |d| > 1024
    logits = (q @ k^T + (q @ w) @ basis^T + u @ k^T + (v @ w) @ basis^T) / 8
    out    = softmax(logits) @ value

Device formulation per core:
  - qT_aug [65,S]: rows 0..63 = (q/8)^T, row 64 = ones.
  - k_augT [65,S]: rows 0..63 = k^T (loaded), row 64 = (u/8) @ k^T (computed).
    qk+uk logits tile = qT_aug_tile^T @ k_augT  (PE, fp32r).
  - Band term: T[i,c'] = qT_aug[:,i] . w2r[:,c'] where c' = j-i+1024 in
    [0,2048], w2r[65, 2176] host-built from w (rows 0..63) and v@w/8 (row 64),
    columns >= 2049 are zero padding.  T is written to DRAM [S rows, pitch
    2176]; the diagonal (rel-shift) read back uses a flat access pattern with
    row step 2175 so each partition p lands j-aligned; out-of-band elements
    read the zero padding of the previous row.
  - Segment A (all 16 tiles up front): band matmuls -> PSUM -> SBUF bf16
    (evac alternating scalar/vector) -> DRAM G_t (gpsimd queue).  Keeps the
    PE stream dense and the scalar/vector engines free for segment C.
  - Segment C per i-tile, quarter-granularity (512 key columns): skewed
    G-read quarter (scalar queue) || qk matmul quarter (fp32r) -> DVE add
    -> ScalarE exp -> P bf16.  No max subtraction (logit range is small).
  - P^T via the DMA XBAR (dma_start_transpose, one call per i-tile with a
    3D out AP scattering 16 [128,128] blocks); XBARs own the sync queue.
  - attn@v lags two tiles (av(t-2)) so its XBAR is long done: out_aug
    [65, 128] = sum_jb [v_jb | 1]^T @ P^T_jb; row 64 accumulates the
    softmax denominators for free.  F(t-3) lags one more: PE transpose of
    the out_aug chunk, 1/denom scale (DVE), store.
"""

import numpy as np
import ml_dtypes

import concourse.bass as bass
import concourse.bacc as bacc
import concourse.mybir as mybir
import concourse.tile as tile
from concourse.bass_utils import run_bass_kernel_spmd
from concourse.masks import make_identity

S = 2048
D = 64
NB = 64          # pos-emb dim (basis features)
H = 8
HALF = NB // 2   # 32
BAND = 1024      # max |d| with nonzero features
Q = S + 128      # G row pitch (2049 band cols + 127 zero pad)
F32 = mybir.dt.float32
F32R = mybir.dt.float32r
BF16 = mybir.dt.bfloat16

_NC_CACHE = {}


def _basis_feature_matrix():
    """Rr[c', n] for c' in [0, Q): features of distance d = 1024 - c'.
    Matches reference._relative_basis numerics (float32)."""
    pow_rate = np.float32(np.exp(np.log((S + 1) / 2) / HALF))
    widths = np.power(pow_rate, np.arange(1, HALF + 1, dtype=np.float32),
                      dtype=np.float32)  # [32]
    d = (np.float32(BAND) - np.arange(Q, dtype=np.float32))[:, None]  # [Q,1]
    unsigned = (np.abs(d) <= widths[None, :]).astype(np.float32)      # [Q,32]
    signed = np.sign(d) * unsigned
    return np.concatenate([unsigned, signed], axis=-1)  # [Q, 64]


def _build_nc():
    if "nc" in _NC_CACHE:
        return _NC_CACHE["nc"]

    nc = bacc.Bacc("TRN2", target_bir_lowering=False, debug=False,
                   num_devices=H)
    d_kT = nc.dram_tensor("kT", [D, S], BF16, kind="ExternalInput")
    d_u8 = nc.dram_tensor("u8", [D, 1], BF16, kind="ExternalInput")
    d_v = nc.dram_tensor("vb", [S, D + 1], BF16, kind="ExternalInput")
    d_w2r = nc.dram_tensor("w2r", [65, Q], BF16, kind="ExternalInput")
    d_qTb = nc.dram_tensor("qT_bf", [65, S], BF16, kind="ExternalInput")
    d_out = nc.dram_tensor("out", [S, D], F32, kind="ExternalOutput")

    NT = S // 128  # 16 i/j tiles
    d_Gs = [nc.dram_tensor(f"gband{t}", [128 * Q], BF16, kind="Internal")
            for t in range(NT)]

    with tile.TileContext(nc) as tc:
        with tc.tile_pool(name="pers", bufs=1) as pers:
            # Queue roles: sync = G writes + output writes (all async
            # issues); scalar = exp + initial loads + share of A evacs;
            # vector/gpsimd = adds, evacs, G-read issues (gpsimd).
            # All matmul operands are zero-padded to K=128 (bf16 K=128
            # streams ~0.5 ns/col vs ~0.84 at K=65).
            sb_qTb = pers.tile([128, S], BF16)
            sb_w2r = pers.tile([128, Q], BF16)
            nc.gpsimd.memset(sb_w2r[64:128, :], 0.0)
            nc.gpsimd.memset(sb_qTb[64:128, :], 0.0)
            nc.scalar.dma_start(out=sb_qTb[0:65, 0:128], in_=d_qTb[:, 0:128])
            # A(0)'s three matmuls need cols [897,1409), [1409,1921),
            # [1921,2176) in that order; A(t) then extends down to
            # clo = 897-128t.  Load in exactly that consumption order.
            for lo, hi in ((897, 1409), (1409, 1921), (1921, Q),
                           (385, 897), (0, 385)):
                nc.scalar.dma_start(out=sb_w2r[0:65, lo:hi],
                                    in_=d_w2r[:, lo:hi])
            nc.scalar.dma_start(out=sb_qTb[0:65, 128:640],
                                in_=d_qTb[:, 128:640])
            nc.scalar.dma_start(out=sb_qTb[0:65, 640:1344],
                                in_=d_qTb[:, 640:1344])
            nc.scalar.dma_start(out=sb_qTb[0:65, 1344:S],
                                in_=d_qTb[:, 1344:S])
            sb_kaug = pers.tile([128, S], BF16)
            nc.gpsimd.memset(sb_kaug[64:128, :], 0.0)
            for c in range(4):
                nc.gpsimd.dma_start(out=sb_kaug[0:D, c * 512:(c + 1) * 512],
                                    in_=d_kT[:, c * 512:(c + 1) * 512])
            sb_v = pers.tile([128, NT, D + 1], BF16)   # [v | ones]
            sb_id = pers.tile([128, 128], F32)
            sb_idb = pers.tile([128, 128], BF16)
            sb_P = pers.tile([128, NT, S], BF16)      # exp(logits), i-tiled
            sb_oT = pers.tile([D + 1, S], F32)        # out^T + denom row
            sb_u8 = pers.tile([D, 1], BF16)
            nc.gpsimd.dma_start(out=sb_u8[:], in_=d_u8[:])

            QW = 512  # quarter width (key columns per qk/exp step)
            with tc.tile_pool(name="gsb", bufs=6) as gsb, \
                 tc.tile_pool(name="bandsb", bufs=3) as bsb, \
                 tc.tile_pool(name="ptsb", bufs=4) as ptsb, \
                 tc.tile_pool(name="fsb", bufs=3) as fsb, \
                 tc.tile_pool(name="ps", bufs=1, space="PSUM") as ps:

                def phase_A(t):
                    i0 = t * 128
                    jlo = max(0, i0 - BAND)
                    jhi = min(S, i0 + 128 + BAND)
                    clo = max(0, (jlo - i0 + BAND) - 127)
                    chi = min(2049, (jhi - 1) - i0 + BAND + 1)
                    gt = gsb.tile([128, Q], BF16)
                    nc.gpsimd.memset(gt[:, chi:Q], 0.0)
                    cuts = list(range(clo, chi, 512)) + [chi]
                    for ci in range(len(cuts) - 1):
                        lo, hi = cuts[ci], cuts[ci + 1]
                        pg = ps.tile([128, 512], F32, tag="c1", bufs=3)
                        nc.tensor.matmul(
                            pg[:, 0:hi - lo],
                            lhsT=sb_qTb[:, i0:i0 + 128],
                            rhs=sb_w2r[:, lo:hi],
                            start=True, stop=True)
                        if (t + ci) % 2 == 0:
                            nc.scalar.copy(out=gt[:, lo:hi],
                                           in_=pg[:, 0:hi - lo])
                        else:
                            nc.vector.tensor_copy(gt[:, lo:hi],
                                                  pg[:, 0:hi - lo])
                    wr = bass.AP(tensor=d_Gs[t], offset=clo,
                                 ap=[[Q, 128], [1, Q - clo]])
                    nc.sync.dma_start(out=wr, in_=gt[:, clo:Q])

                def phase_Crd(t):
                    # Skewed G read for tile t (single wide DMA: 128
                    # descriptors of up to 4.3KB); issued from the gpsimd
                    # queue 2 tiles ahead of use so the transfer hides
                    # under two tiles of exp work.
                    i0 = t * 128
                    jlo = max(0, i0 - BAND)
                    jhi = min(S, i0 + 128 + BAND)
                    bt = bsb.tile([128, Q], BF16)
                    wdt = jhi - jlo
                    rd = bass.AP(tensor=d_Gs[t], offset=(jlo - i0 + BAND),
                                 ap=[[Q - 1, 128], [1, wdt]])
                    nc.gpsimd.dma_start(out=bt[:, 0:wdt], in_=rd)
                    return bt

                def phase_C(t, bt):
                    i0 = t * 128
                    jlo = max(0, i0 - BAND)
                    jhi = min(S, i0 + 128 + BAND)
                    for qtr in range(4):
                        j0 = qtr * QW
                        pq = ps.tile([128, QW], F32, tag="qk", bufs=4)
                        nc.tensor.matmul(
                            pq[:],
                            lhsT=sb_qTb[:, i0:i0 + 128],
                            rhs=sb_kaug[:, j0:j0 + QW],
                            start=True, stop=True)
                        alo = max(jlo, j0)
                        ahi = min(jhi, j0 + QW)
                        if alo < ahi:
                            nc.vector.tensor_add(
                                pq[:, alo - j0:ahi - j0],
                                pq[:, alo - j0:ahi - j0],
                                bt[:, alo - jlo:ahi - jlo])
                        nc.scalar.activation(
                            out=sb_P[:, t, j0:j0 + QW], in_=pq[:],
                            func=mybir.ActivationFunctionType.Exp)

                po_live = {}

                def phase_E_sub(c, sub):
                    # Quarter of chunk c's attn@v: PE-transpose P blocks
                    # for key-block range [4*sub, 4*sub+4) (bf16 PSUM out),
                    # evacuate (alternating scalar/vector), accumulate
                    # out_aug[65, 512] over the 16 key blocks.
                    if sub == 0:
                        po_live[c] = ps.tile([D + 1, 512], F32, tag="po",
                                             bufs=1, name=f"po{c}")
                    po = po_live[c]
                    for jb in range(4 * sub, 4 * sub + 4):
                        ptp = ps.tile([128, 512], BF16, tag="c1", bufs=3)
                        for s in range(4):
                            t = 4 * c + s
                            nc.tensor.transpose(
                                ptp[:, s * 128:(s + 1) * 128],
                                sb_P[:, t, jb * 128:(jb + 1) * 128],
                                sb_idb[:])
                        pt = ptsb.tile([128, 512], BF16)
                        if jb % 2 == 0:
                            nc.scalar.copy(out=pt[:], in_=ptp[:])
                        else:
                            nc.vector.tensor_copy(pt[:], ptp[:])
                        nc.tensor.matmul(po[:], lhsT=sb_v[:, jb, :],
                                         rhs=pt[:],
                                         start=(jb == 0), stop=(jb == NT - 1))
                    if sub == 3:
                        nc.vector.tensor_copy(
                            sb_oT[:, c * 512:(c + 1) * 512],
                            po_live.pop(c))

                def phase_F(t):
                    # pf borrows a "qk" slot (same 2KB size) so the po
                    # accumulator bank is never blocked by output fixup.
                    pf = ps.tile([128, 512], F32, tag="qk", bufs=4)
                    nc.tensor.transpose(pf[:, 0:D + 1],
                                        sb_oT[:, t * 128:(t + 1) * 128],
                                        sb_id[0:D + 1, 0:D + 1])
                    rc = fsb.tile([128, 1], F32, tag="rc")
                    nc.vector.reciprocal(rc[:], pf[:, D:D + 1])
                    ot = fsb.tile([128, D], F32, tag="ot")
                    nc.vector.tensor_scalar_mul(ot[:], pf[:, 0:D], rc[:])
                    nc.sync.dma_start(out=d_out[t * 128:(t + 1) * 128, :],
                                      in_=ot[:])

                # ---- Segment A: all band tiles up front (dense PE);
                #      first two G reads prefetched mid-segment. ----
                bts = {}
                for t in range(NT):
                    phase_A(t)
                    if t == 2:
                        bts[0] = phase_Crd(0)
                    if t == 3:
                        bts[1] = phase_Crd(1)
                # Deferred setup (needed from C(0)/E(0) on).
                for c in range(4):
                    pk = ps.tile([128, 512], F32, tag="c1", bufs=3)
                    nc.tensor.matmul(
                        pk[0:1, :],
                        lhsT=sb_u8[:],
                        rhs=sb_kaug[0:D, c * 512:(c + 1) * 512],
                        start=True, stop=True)
                    nc.scalar.copy(
                        out=sb_kaug[64:65, c * 512:(c + 1) * 512],
                        in_=pk[0:1, :])
                for t in range(NT):
                    nc.gpsimd.dma_start(out=sb_v[:, t, :],
                                        in_=d_v[t * 128:(t + 1) * 128, :])
                make_identity(nc, sb_id[:])
                make_identity(nc, sb_idb[:])

                # ---- Segment C: qk + band add + exp per tile; attn@v
                #      (E) spread one key-quarter per iteration, lagging
                #      4-7 tiles behind its chunk's exps; F per tile. ----
                for t in range(NT):
                    if t + 2 < NT:
                        bts[t + 2] = phase_Crd(t + 2)
                    phase_C(t, bts.pop(t))
                    if t - 4 >= 0:
                        phase_E_sub((t - 4) // 4, (t - 4) % 4)
                    if t - 8 >= 0:
                        phase_F(t - 8)
                for k in range(4):
                    phase_E_sub(3, k)
                    phase_F(NT - 8 + k)
                for t in range(NT - 4, NT):
                    phase_F(t)

    nc.finalize()
    _NC_CACHE["nc"] = nc
    return nc


def _host_prep(query, key, value, u, v, w):
    """Build the 8 per-core input maps from the full inputs."""
    q = np.asarray(query, np.float32)[0]   # [S,H,D]
    k = np.asarray(key, np.float32)[0]
    val = np.asarray(value, np.float32)[0]
    u = np.asarray(u, np.float32)
    v = np.asarray(v, np.float32)
    w = np.asarray(w, np.float32)
    Rr = _basis_feature_matrix()           # [Q, 64]

    ones = np.ones((1, S), np.float32)
    in_maps = []
    for h in range(H):
        qT8 = np.ascontiguousarray(q[:, h, :].T) / np.float32(8.0)  # [64,S]
        qT_aug = np.concatenate([qT8, ones], axis=0)                # [65,S]
        kT = np.ascontiguousarray(k[:, h, :].T)                     # [64,S]
        u8 = (u[h] / np.float32(8.0)).reshape(D, 1)
        vb = np.concatenate([val[:, h, :], ones.T], axis=1)         # [S,65]
        vb = vb.astype(ml_dtypes.bfloat16)
        w2r_qr = w[h] @ Rr.T                                        # [64,Q]
        vw8 = (v[h] @ w[h]) / np.float32(8.0)                       # [64]
        w2r_vr = (vw8 @ Rr.T).reshape(1, Q)                         # [1,Q]
        w2r = np.concatenate([w2r_qr, w2r_vr],
                             axis=0).astype(ml_dtypes.bfloat16)
        in_maps.append({
            "qT_bf": np.ascontiguousarray(qT_aug).astype(ml_dtypes.bfloat16),
            "kT": kT.astype(ml_dtypes.bfloat16),
            "u8": np.ascontiguousarray(u8).astype(ml_dtypes.bfloat16),
            "vb": np.ascontiguousarray(vb),
            "w2r": np.ascontiguousarray(w2r),
        })
    return in_maps


def kernel(query, key, value, u, v, w, _trace=False):
    nc = _build_nc()
    in_maps = _host_prep(query, key, value, u, v, w)
    res = run_bass_kernel_spmd(nc, in_maps, core_ids=list(range(H)),
                               trace=_trace)
    outs = np.stack([res.results[h]["out"] for h in range(H)])  # [H,S,D]
    full = np.transpose(outs, (1, 0, 2))[None]                  # [1,S,H,D]
    out = np.ascontiguousarray(full.astype(np.float32))
    if _trace:
        return out, res
    return out


if __name__ == "__main__":
    rng = np.random.default_rng(0)
    ins = {
        "query": rng.standard_normal((1, S, H, D), np.float32),
        "key": rng.standard_normal((1, S, H, D), np.float32),
        "value": rng.standard_normal((1, S, H, D), np.float32),
        "u": rng.standard_normal((H, D), np.float32),
        "v": rng.standard_normal((H, D), np.float32),
        "w": rng.standard_normal((H, D, NB), np.float32),
    }
    out = kernel(**ins)
    print("out shape:", out.shape, "finite:", np.isfinite(out).all())


# revision 38
# speedup vs baseline: 1.1432x; 1.0746x over previous
"""Enformer dot-product self-attention with central-mask relative position
bias, on 8 Trainium2 NeuronCores (one head per core, SPMD).

Math per head h (S=2048, D=64, N=64):
    basis[i,j,:] = f(d=i-j)  — indicator features, zero for |d| > 1024
    logits = (q @ k^T + (q @ w) @ basis^T + u @ k^T + (v @ w) @ basis^T) / 8
    out    = softmax(logits) @ value

Device formulation per core:
  - qT_aug [65,S]: rows 0..63 = (q/8)^T, row 64 = ones.
  - k_augT [65,S]: rows 0..63 = k^T (loaded), row 64 = (u/8) @ k^T (computed).
    qk+uk logits tile = qT_aug_tile^T @ k_augT  (PE, fp32r).
  - Band term: T[i,c'] = qT_aug[:,i] . w2r[:,c'] where c' = j-i+1024 in
    [0,2048], w2r[65, 2176] host-built from w (rows 0..63) and v@w/8 (row 64),
    columns >= 2049 are zero padding.  T is written to DRAM [S rows, pitch
    2176]; the diagonal (rel-shift) read back uses a flat access pattern with
    row step 2175 so each partition p lands j-aligned; out-of-band elements
    read the zero padding of the previous row.
  - Segment A (all 16 tiles up front): band matmuls -> PSUM -> SBUF bf16
    (evac alternating scalar/vector) -> DRAM G_t (gpsimd queue).  Keeps the
    PE stream dense and the scalar/vector engines free for segment C.
  - Segment C per i-tile, quarter-granularity (512 key columns): skewed
    G-read quarter (scalar queue) || qk matmul quarter (fp32r) -> DVE add
    -> ScalarE exp -> P bf16.  No max subtraction (logit range is small).
  - P^T via the DMA XBAR (dma_start_transpose, one call per i-tile with a
    3D out AP scattering 16 [128,128] blocks); XBARs own the sync queue.
  - attn@v lags two tiles (av(t-2)) so its XBAR is long done: out_aug
    [65, 128] = sum_jb [v_jb | 1]^T @ P^T_jb; row 64 accumulates the
    softmax denominators for free.  F(t-3) lags one more: PE transpose of
    the out_aug chunk, 1/denom scale (DVE), store.
"""

import numpy as np
import ml_dtypes

import concourse.bass as bass
import concourse.bacc as bacc
import concourse.mybir as mybir
import concourse.tile as tile
from concourse.bass_utils import run_bass_kernel_spmd
from concourse.masks import make_identity

S = 2048
D = 64
NB = 64          # pos-emb dim (basis features)
H = 8
HALF = NB // 2   # 32
BAND = 1024      # max |d| with nonzero features
Q = S + 128      # G row pitch (2049 band cols + 127 zero pad)
F32 = mybir.dt.float32
F32R = mybir.dt.float32r
BF16 = mybir.dt.bfloat16

_NC_CACHE = {}


def _basis_feature_matrix():
    """Rr[c', n] for c' in [0, Q): features of distance d = 1024 - c'.
    Matches reference._relative_basis numerics (float32)."""
    pow_rate = np.float32(np.exp(np.log((S + 1) / 2) / HALF))
    widths = np.power(pow_rate, np.arange(1, HALF + 1, dtype=np.float32),
                      dtype=np.float32)  # [32]
    d = (np.float32(BAND) - np.arange(Q, dtype=np.float32))[:, None]  # [Q,1]
    unsigned = (np.abs(d) <= widths[None, :]).astype(np.float32)      # [Q,32]
    signed = np.sign(d) * unsigned
    return np.concatenate([unsigned, signed], axis=-1)  # [Q, 64]


def _build_nc():
    if "nc" in _NC_CACHE:
        return _NC_CACHE["nc"]

    nc = bacc.Bacc("TRN2", target_bir_lowering=False, debug=False,
                   num_devices=H)
    d_kT = nc.dram_tensor("kT", [D, S], BF16, kind="ExternalInput")
    d_u8 = nc.dram_tensor("u8", [D, 1], BF16, kind="ExternalInput")
    d_v = nc.dram_tensor("vb", [S, D + 1], BF16, kind="ExternalInput")
    d_w2r = nc.dram_tensor("w2r", [128, Q], BF16, kind="ExternalInput")
    d_qTb = nc.dram_tensor("qT_bf", [128, S], BF16, kind="ExternalInput")
    d_out = nc.dram_tensor("out", [S, D], F32, kind="ExternalOutput")

    NT = S // 128  # 16 i/j tiles
    d_Gs = [nc.dram_tensor(f"gband{t}", [128 * Q], BF16, kind="Internal")
            for t in range(NT)]

    with tile.TileContext(nc) as tc:
        with tc.tile_pool(name="pers", bufs=1) as pers:
            # Queue roles: sync = G writes + output writes (all async
            # issues); scalar = exp + initial loads + share of A evacs;
            # vector/gpsimd = adds, evacs, G-read issues (gpsimd).
            # All matmul operands are zero-padded to K=128 (bf16 K=128
            # streams ~0.5 ns/col vs ~0.84 at K=65).
            sb_qTb = pers.tile([128, S], BF16)
            sb_w2r = pers.tile([128, Q], BF16)
            nc.scalar.dma_start(out=sb_qTb[:, 0:128], in_=d_qTb[:, 0:128])
            # A(0)'s three matmuls need cols [897,1409), [1409,1921),
            # [1921,2176) in that order; A(t) then extends down to
            # clo = 897-128t.  Load in exactly that consumption order.
            for lo, hi in ((897, 1409), (1409, 1921), (1921, Q),
                           (385, 897), (0, 385)):
                nc.scalar.dma_start(out=sb_w2r[:, lo:hi],
                                    in_=d_w2r[:, lo:hi])
            nc.scalar.dma_start(out=sb_qTb[:, 128:640],
                                in_=d_qTb[:, 128:640])
            nc.scalar.dma_start(out=sb_qTb[:, 640:1344],
                                in_=d_qTb[:, 640:1344])
            nc.scalar.dma_start(out=sb_qTb[:, 1344:S],
                                in_=d_qTb[:, 1344:S])
            sb_kaug = pers.tile([128, S], BF16)
            nc.gpsimd.memset(sb_kaug[64:128, :], 0.0)
            for c in range(4):
                nc.sync.dma_start(out=sb_kaug[0:D, c * 512:(c + 1) * 512],
                                  in_=d_kT[:, c * 512:(c + 1) * 512])
            sb_v = pers.tile([128, NT, D + 1], BF16)   # [v | ones]
            sb_id = pers.tile([128, 128], F32)
            sb_idb = pers.tile([128, 128], BF16)
            sb_P = pers.tile([128, NT, S], BF16)      # exp(logits), i-tiled
            sb_oT = pers.tile([D + 1, S], F32)        # out^T + denom row
            sb_u8 = pers.tile([D, 1], BF16)
            nc.gpsimd.dma_start(out=sb_u8[:], in_=d_u8[:])

            QW = 512  # quarter width (key columns per qk/exp step)
            with tc.tile_pool(name="gsb", bufs=6) as gsb, \
                 tc.tile_pool(name="bandsb", bufs=3) as bsb, \
                 tc.tile_pool(name="ptsb", bufs=4) as ptsb, \
                 tc.tile_pool(name="fsb", bufs=3) as fsb, \
                 tc.tile_pool(name="ps", bufs=1, space="PSUM") as ps:

                def phase_A(t):
                    i0 = t * 128
                    jlo = max(0, i0 - BAND)
                    jhi = min(S, i0 + 128 + BAND)
                    clo = max(0, (jlo - i0 + BAND) - 127)
                    chi = min(2049, (jhi - 1) - i0 + BAND + 1)
                    gt = gsb.tile([128, Q], BF16)
                    nc.gpsimd.memset(gt[:, chi:Q], 0.0)
                    cuts = list(range(clo, chi, 512)) + [chi]
                    for ci in range(len(cuts) - 1):
                        lo, hi = cuts[ci], cuts[ci + 1]
                        pg = ps.tile([128, 512], F32, tag="c1", bufs=3)
                        nc.tensor.matmul(
                            pg[:, 0:hi - lo],
                            lhsT=sb_qTb[:, i0:i0 + 128],
                            rhs=sb_w2r[:, lo:hi],
                            start=True, stop=True)
                        if (t + ci) % 2 == 0:
                            nc.scalar.copy(out=gt[:, lo:hi],
                                           in_=pg[:, 0:hi - lo])
                        else:
                            nc.vector.tensor_copy(gt[:, lo:hi],
                                                  pg[:, 0:hi - lo])
                    wr = bass.AP(tensor=d_Gs[t], offset=clo,
                                 ap=[[Q, 128], [1, Q - clo]])
                    nc.sync.dma_start(out=wr, in_=gt[:, clo:Q])

                def phase_Crd(t):
                    # Skewed G read for tile t (single wide DMA: 128
                    # descriptors of up to 4.3KB); issued from the gpsimd
                    # queue 2 tiles ahead of use so the transfer hides
                    # under two tiles of exp work.
                    i0 = t * 128
                    jlo = max(0, i0 - BAND)
                    jhi = min(S, i0 + 128 + BAND)
                    bt = bsb.tile([128, Q], BF16)
                    wdt = jhi - jlo
                    rd = bass.AP(tensor=d_Gs[t], offset=(jlo - i0 + BAND),
                                 ap=[[Q - 1, 128], [1, wdt]])
                    nc.gpsimd.dma_start(out=bt[:, 0:wdt], in_=rd)
                    return bt

                def phase_C(t, bt):
                    i0 = t * 128
                    jlo = max(0, i0 - BAND)
                    jhi = min(S, i0 + 128 + BAND)
                    for qtr in range(4):
                        j0 = qtr * QW
                        pq = ps.tile([128, QW], F32, tag="qk", bufs=4)
                        nc.tensor.matmul(
                            pq[:],
                            lhsT=sb_qTb[:, i0:i0 + 128],
                            rhs=sb_kaug[:, j0:j0 + QW],
                            start=True, stop=True)
                        alo = max(jlo, j0)
                        ahi = min(jhi, j0 + QW)
                        if alo < ahi:
                            nc.vector.tensor_add(
                                pq[:, alo - j0:ahi - j0],
                                pq[:, alo - j0:ahi - j0],
                                bt[:, alo - jlo:ahi - jlo])
                        nc.scalar.activation(
                            out=sb_P[:, t, j0:j0 + QW], in_=pq[:],
                            func=mybir.ActivationFunctionType.Exp)

                po_live = {}

                def phase_E_sub(c, sub):
                    # Quarter of chunk c's attn@v: PE-transpose P blocks
                    # for key-block range [4*sub, 4*sub+4) (bf16 PSUM out),
                    # evacuate (alternating scalar/vector), accumulate
                    # out_aug[65, 512] over the 16 key blocks.
                    if sub == 0:
                        po_live[c] = ps.tile([D + 1, 512], F32, tag="po",
                                             bufs=1, name=f"po{c}")
                    po = po_live[c]
                    for jb in range(4 * sub, 4 * sub + 4):
                        ptp = ps.tile([128, 512], BF16, tag="c1", bufs=3)
                        for s in range(4):
                            t = 4 * c + s
                            nc.tensor.transpose(
                                ptp[:, s * 128:(s + 1) * 128],
                                sb_P[:, t, jb * 128:(jb + 1) * 128],
                                sb_idb[:])
                        pt = ptsb.tile([128, 512], BF16)
                        if jb % 8 in (0, 3, 6):
                            nc.scalar.copy(out=pt[:], in_=ptp[:])
                        else:
                            nc.vector.tensor_copy(pt[:], ptp[:])
                        nc.tensor.matmul(po[:], lhsT=sb_v[:, jb, :],
                                         rhs=pt[:],
                                         start=(jb == 0), stop=(jb == NT - 1))
                    if sub == 3:
                        nc.vector.tensor_copy(
                            sb_oT[:, c * 512:(c + 1) * 512],
                            po_live.pop(c))

                def phase_F(t):
                    # pf borrows a "qk" slot (same 2KB size) so the po
                    # accumulator bank is never blocked by output fixup.
                    pf = ps.tile([128, 512], F32, tag="qk", bufs=4)
                    nc.tensor.transpose(pf[:, 0:D + 1],
                                        sb_oT[:, t * 128:(t + 1) * 128],
                                        sb_id[0:D + 1, 0:D + 1])
                    rc = fsb.tile([128, 1], F32, tag="rc")
                    nc.vector.reciprocal(rc[:], pf[:, D:D + 1])
                    ot = fsb.tile([128, D], F32, tag="ot")
                    nc.vector.tensor_scalar_mul(ot[:], pf[:, 0:D], rc[:])
                    nc.sync.dma_start(out=d_out[t * 128:(t + 1) * 128, :],
                                      in_=ot[:])

                # ---- Segment A: all band tiles up front (dense PE);
                #      first two G reads prefetched mid-segment. ----
                bts = {}
                for t in range(NT):
                    phase_A(t)
                    if t == 2:
                        bts[0] = phase_Crd(0)
                    if t == 3:
                        bts[1] = phase_Crd(1)
                # Deferred setup (needed from C(0)/E(0) on).
                for c in range(4):
                    pk = ps.tile([128, 512], F32, tag="c1", bufs=3)
                    nc.tensor.matmul(
                        pk[0:1, :],
                        lhsT=sb_u8[:],
                        rhs=sb_kaug[0:D, c * 512:(c + 1) * 512],
                        start=True, stop=True)
                    nc.scalar.copy(
                        out=sb_kaug[64:65, c * 512:(c + 1) * 512],
                        in_=pk[0:1, :])
                for t in range(NT):
                    nc.sync.dma_start(out=sb_v[:, t, :],
                                      in_=d_v[t * 128:(t + 1) * 128, :])
                make_identity(nc, sb_id[:])
                make_identity(nc, sb_idb[:])

                # ---- Segment C: qk + band add + exp per tile; attn@v
                #      (E) spread one key-quarter per iteration, lagging
                #      4-7 tiles behind its chunk's exps; F per tile. ----
                for t in range(NT):
                    if t + 2 < NT:
                        bts[t + 2] = phase_Crd(t + 2)
                    phase_C(t, bts.pop(t))
                    if t - 4 >= 0:
                        phase_E_sub((t - 4) // 4, (t - 4) % 4)
                    if t - 8 >= 0:
                        phase_F(t - 8)
                for k in range(4):
                    phase_E_sub(3, k)
                    phase_F(NT - 8 + k)
                for t in range(NT - 4, NT):
                    phase_F(t)

    nc.finalize()
    _NC_CACHE["nc"] = nc
    return nc


def _host_prep(query, key, value, u, v, w):
    """Build the 8 per-core input maps from the full inputs."""
    q = np.asarray(query, np.float32)[0]   # [S,H,D]
    k = np.asarray(key, np.float32)[0]
    val = np.asarray(value, np.float32)[0]
    u = np.asarray(u, np.float32)
    v = np.asarray(v, np.float32)
    w = np.asarray(w, np.float32)
    Rr = _basis_feature_matrix()           # [Q, 64]

    ones = np.ones((1, S), np.float32)
    in_maps = []
    for h in range(H):
        qT8 = np.ascontiguousarray(q[:, h, :].T) / np.float32(8.0)  # [64,S]
        qT_aug = np.concatenate([qT8, ones], axis=0)                # [65,S]
        kT = np.ascontiguousarray(k[:, h, :].T)                     # [64,S]
        u8 = (u[h] / np.float32(8.0)).reshape(D, 1)
        vb = np.concatenate([val[:, h, :], ones.T], axis=1)         # [S,65]
        vb = vb.astype(ml_dtypes.bfloat16)
        w2r_qr = w[h] @ Rr.T                                        # [64,Q]
        vw8 = (v[h] @ w[h]) / np.float32(8.0)                       # [64]
        w2r_vr = (vw8 @ Rr.T).reshape(1, Q)                         # [1,Q]
        w2r = np.concatenate(
            [w2r_qr, w2r_vr, np.zeros((63, Q), np.float32)],
            axis=0).astype(ml_dtypes.bfloat16)
        qT_pad = np.concatenate([qT_aug, np.zeros((63, S), np.float32)],
                                axis=0)
        in_maps.append({
            "qT_bf": np.ascontiguousarray(qT_pad).astype(ml_dtypes.bfloat16),
            "kT": kT.astype(ml_dtypes.bfloat16),
            "u8": np.ascontiguousarray(u8).astype(ml_dtypes.bfloat16),
            "vb": np.ascontiguousarray(vb),
            "w2r": np.ascontiguousarray(w2r),
        })
    return in_maps


def kernel(query, key, value, u, v, w, _trace=False):
    nc = _build_nc()
    in_maps = _host_prep(query, key, value, u, v, w)
    res = run_bass_kernel_spmd(nc, in_maps, core_ids=list(range(H)),
                               trace=_trace)
    outs = np.stack([res.results[h]["out"] for h in range(H)])  # [H,S,D]
    full = np.transpose(outs, (1, 0, 2))[None]                  # [1,S,H,D]
    out = np.ascontiguousarray(full.astype(np.float32))
    if _trace:
        return out, res
    return out


if __name__ == "__main__":
    rng = np.random.default_rng(0)
    ins = {
        "query": rng.standard_normal((1, S, H, D), np.float32),
        "key": rng.standard_normal((1, S, H, D), np.float32),
        "value": rng.standard_normal((1, S, H, D), np.float32),
        "u": rng.standard_normal((H, D), np.float32),
        "v": rng.standard_normal((H, D), np.float32),
        "w": rng.standard_normal((H, D, NB), np.float32),
    }
    out = kernel(**ins)
    print("out shape:", out.shape, "finite:", np.isfinite(out).all())
